# revision 12
# baseline (speedup 1.0000x reference)
"""Trainium2 Bass kernel for 6-layer CROWN backsubstitution — Strassen variant.

Same structure as kernel.py (row-sharded transposed state, masked SA/SB
tiles, bf16 matmuls, fp32 PSUM) but composition iterations in STRSET use
a single-level Strassen split of the [2048, 4096] @ [4096, 512] block
product: 2x2 blocks over (out-rows x contraction x state-cols), 7
M-products of [1024, 2048] @ [2048, 256] instead of 8 -> 12.5% fewer PE
cycles. A-operand combos are precomputed on host (free); B-combos are 5
elementwise bf16 tile sets computed on DVE; C-assembly runs on ACT/DVE
against the one-PSUM-operand rule.
"""

import ml_dtypes
import numpy as np

import concourse.bass as bass
import concourse.bacc as bacc
import concourse.tile as tile
from concourse import mybir
from concourse.bass_utils import run_bass_kernel_spmd

L = 6
D = 2048
N_CORES = 8
MSH = D // N_CORES          # 256 rows per core
KT = D // 128               # 16 k-tiles
NSWEEP = L
NITER = L - 1
STRSET = frozenset({0, 1, 2, 3, 4})   # iterations using Strassen

F32 = mybir.dt.float32
BF16 = mybir.dt.bfloat16
NPBF16 = ml_dtypes.bfloat16
SUB = mybir.AluOpType.subtract

LAST_RESULTS = None
_CACHED_NC = None


def _build_nc(bench_reps=1, strset=STRSET, wch_bufs=10, combo_bufs=1,
              gps_combos=False):
    strs = sorted(strset)
    stds = [t for t in range(NITER) if t not in strset]
    nc = bacc.Bacc("TRN2", target_bir_lowering=False, debug=False,
                   num_devices=N_CORES)

    # Standard-iteration weights (chunk for (t_pos, m) at rows
    # [(t_pos*KT + m)*128, +128)); dummy 1-chunk if unused.
    wq_rows = max(len(stds), 1) * KT * 128
    WQ = nc.dram_tensor("WQ", [wq_rows, KT * 2 * 128], BF16,
                        kind="ExternalInput").ap()
    # Strassen weights: row block (s*8 + mt)*128, col (op*16 + k)*128 + c
    wqs_rows = max(len(strs), 1) * 8 * 128
    WQS = nc.dram_tensor("WQS", [wqs_rows, 7 * KT * 128], BF16,
                         kind="ExternalInput").ap()
    SAB0 = nc.dram_tensor("SAB0", [2, KT, 128, 512], BF16,
                          kind="ExternalInput").ap()
    VECS = nc.dram_tensor("VECS", [128, NSWEEP * 2 * KT * 2], BF16,
                          kind="ExternalInput").ap()
    BI = nc.dram_tensor("BI", [2, 512], F32, kind="ExternalInput").ap()
    OUT = nc.dram_tensor("OUT", [1, 512], F32, kind="ExternalOutput").ap()

    with tile.TileContext(nc) as tc:
        with (
            tc.tile_pool(name="state", bufs=1) as state_pool,
            tc.tile_pool(name="wchunk", bufs=wch_bufs) as wpool,
            tc.tile_pool(name="combo", bufs=combo_bufs) as cpool,
            tc.tile_pool(name="small", bufs=1) as spool,
            tc.tile_pool(name="tmp", bufs=2) as tpool,
            tc.tile_pool(name="asm", bufs=3) as apool,
            tc.tile_pool(name="qpsum", bufs=7, space="PSUM") as qpool,
            tc.tile_pool(name="vpsum", bufs=1, space="PSUM") as vpool,
        ):
            if bench_reps > 1:
                loop_ctx = tc.For_i(0, bench_reps, 1)
                loop_ctx.__enter__()

            SA = [[state_pool.tile([128, 512], BF16, tag=f"sa{s}_{k}",
                                   name=f"sa{s}_{k}")
                   for k in range(KT)] for s in range(2)]
            SB = [[state_pool.tile([128, 512], BF16, tag=f"sb{s}_{k}",
                                   name=f"sb{s}_{k}")
                   for k in range(KT)] for s in range(2)]

            vecs = spool.tile([128, NSWEEP * 2 * KT * 2], BF16, tag="vecs")
            nc.scalar.dma_start(vecs[:], VECS[:])
            best = spool.tile([1, 512], F32, tag="best")
            nc.scalar.dma_start(best[:], BI[0:1, :])
            bb = spool.tile([1, 512], F32, tag="bb")
            nc.scalar.dma_start(bb[:], BI[1:2, :])

            # State load split across queues; the first Strassen mt's weight
            # chunks are interleaved mid-state so the first chains don't
            # wait on a queue-serialized 4MB state load. (Loaded once, also
            # reused read-only by every bench_reps iteration.)
            pre_w = {}
            for k in range(KT // 2):
                nc.sync.dma_start(SA[0][k][:], SAB0[0, k])
                nc.scalar.dma_start(SB[0][k][:], SAB0[1, k])
            if strs and strs[0] == 0:
                lst = []
                for op in range(7):
                    w = wpool.tile([128, KT * 128], BF16, tag="wop",
                                   name=f"w_pre_{op}")
                    eng = nc.sync if op % 2 == 0 else nc.scalar
                    eng.dma_start(
                        w[:], WQS[0:128, op * KT * 128:(op + 1) * KT * 128])
                    lst.append(w)
                pre_w[(0, 0)] = lst
            for k in range(KT // 2, KT):
                nc.sync.dma_start(SA[0][k][:], SAB0[0, k])
                nc.scalar.dma_start(SB[0][k][:], SAB0[1, k])

            def sweep(vbase, cur, update_bb, nm):
                """Concretization sweep, 4-way PE column tiling."""
                pv = vpool.tile([128, 512], F32, tag="pv", name=f"pv_{nm}")
                for s in range(8):
                    for g in range(4):
                        side = SA if g < 2 else SB
                        ab = 0 if g < 2 else 1
                        k = (g % 2) * 8 + s
                        j = vbase + (ab * KT + k) * 2
                        nc.tensor.matmul(pv[32 * g:32 * g + 2, :],
                                         vecs[:, j:j + 2], side[cur][k][:],
                                         start=(s == 0), stop=(s == 7),
                                         tile_position=(0, 32 * g))
                s2 = tpool.tile([2, 512], F32, tag="s2", name=f"s2_{nm}")
                nc.scalar.copy(s2[:], pv[0:2, :])
                nc.vector.tensor_add(s2[:], s2[:], pv[32:34, :])
                nc.vector.tensor_add(s2[:], s2[:], pv[64:66, :])
                nc.vector.tensor_add(s2[:], s2[:], pv[96:98, :])
                curt = tpool.tile([1, 512], F32, tag="curt", name=f"curt_{nm}")
                nc.vector.tensor_add(curt[:], s2[0:1, :], bb[:])
                nc.vector.tensor_tensor(best[:], best[:], curt[:],
                                        op=mybir.AluOpType.min)
                if update_bb:
                    bdel = tpool.tile([1, 512], F32, tag="bdel",
                                      name=f"bdel_{nm}")
                    nc.scalar.dma_start(bdel[:], s2[1:2, :])
                    nc.vector.tensor_add(bb[:], bb[:], bdel[:])

            def comp_std(t_pos, cur, nxt, nm):
                """Standard composition: 16 chains of 32 N=512 matmuls."""
                row_base = t_pos * KT * 128
                for m in range(KT):
                    wch = wpool.tile([128, KT * 2 * 128], BF16, tag="wch",
                                     name=f"wch_{nm}_{m}")
                    eng = nc.sync if m % 2 == 0 else nc.scalar
                    eng.dma_start(wch[:],
                                  WQ[row_base + m * 128:row_base + (m + 1) * 128, :])
                    ps = qpool.tile([128, 512], F32, tag="qp512",
                                    name=f"qp_{nm}_{m}")
                    for k in range(KT):
                        nc.tensor.matmul(
                            ps[:], wch[:, (2 * k) * 128:(2 * k + 1) * 128],
                            SA[cur][k][:], start=(k == 0), stop=False)
                        nc.tensor.matmul(
                            ps[:], wch[:, (2 * k + 1) * 128:(2 * k + 2) * 128],
                            SB[cur][k][:], start=False, stop=(k == KT - 1))
                    nc.scalar.activation(SA[nxt][m][:], ps[:],
                                         mybir.ActivationFunctionType.Relu)
                    nc.vector.tensor_scalar_min(SB[nxt][m][:], ps[:], 0.0)

            def comp_strassen(s_idx, cur, nxt, nm):
                """Strassen composition: per mt, 7 chains of 16 N=256 matmuls.

                B-blocks (bf16): B11=SA[:, :256], B12=SA[:, 256:],
                B21=SB[:, :256], B22=SB[:, 256:].
                """
                # B-combos on DVE, one op per (type, k).
                CB = {}
                for cn, fa, fb, op in (
                    ("c1", lambda k: SA[cur][k][:, 0:256],
                     lambda k: SB[cur][k][:, 256:512], None),      # B11+B22
                    ("c3", lambda k: SA[cur][k][:, 256:512],
                     lambda k: SB[cur][k][:, 256:512], SUB),       # B12-B22
                    ("c4", lambda k: SB[cur][k][:, 0:256],
                     lambda k: SA[cur][k][:, 0:256], SUB),         # B21-B11
                    ("c6", lambda k: SA[cur][k][:, 0:256],
                     lambda k: SA[cur][k][:, 256:512], None),      # B11+B12
                    ("c7", lambda k: SB[cur][k][:, 0:256],
                     lambda k: SB[cur][k][:, 256:512], None),      # B21+B22
                ):
                    tiles = []
                    for k in range(KT):
                        cb = cpool.tile([128, 256], BF16, tag=f"{cn}_{k}",
                                        name=f"{cn}_{nm}_{k}")
                        if op is None:
                            eng = (nc.gpsimd if (gps_combos and
                                                 cn in ("c6", "c7"))
                                   else nc.vector)
                            eng.tensor_add(cb[:], fa(k), fb(k))
                        else:
                            nc.vector.tensor_tensor(cb[:], fa(k), fb(k), op=op)
                        tiles.append(cb)
                    CB[cn] = tiles

                def rhs(op, k):
                    return (CB["c1"][k][:], SA[cur][k][:, 0:256],
                            CB["c3"][k][:], CB["c4"][k][:],
                            SB[cur][k][:, 256:512], CB["c6"][k][:],
                            CB["c7"][k][:])[op]

                row0 = s_idx * 8 * 128
                for mt in range(8):
                    wops = pre_w.get((s_idx, mt))
                    if wops is None:
                        wops = []
                        for op in range(7):
                            w = wpool.tile([128, KT * 128], BF16, tag="wop",
                                           name=f"w_{nm}_{mt}_{op}")
                            eng = (nc.sync if (mt * 7 + op) % 2 == 0
                                   else nc.scalar)
                            r = row0 + mt * 128
                            eng.dma_start(
                                w[:], WQS[r:r + 128,
                                          op * KT * 128:(op + 1) * KT * 128])
                            wops.append(w)
                    # Two M-products per PSUM bank (disjoint column halves,
                    # sequential accumulation groups).
                    psp = [qpool.tile([128, 512], F32, tag="qp512",
                                      name=f"mp_{nm}_{mt}_{j}")
                           for j in range(4)]
                    ps = [psp[i // 2][:, (i % 2) * 256:(i % 2 + 1) * 256]
                          for i in range(7)]
                    for op in range(7):
                        for k in range(KT):
                            nc.tensor.matmul(ps[op],
                                             wops[op][:, k * 128:(k + 1) * 128],
                                             rhs(op, k), start=(k == 0),
                                             stop=(k == KT - 1))
                    # Assembly: C11=M1+M4-M5+M7  C12=M3+M5  C21=M2+M4
                    #           C22=M1-M2+M3+M6      (ps[i] = M_{i+1})
                    m1c = apool.tile([128, 256], F32, tag="m1c",
                                     name=f"m1c_{nm}_{mt}")
                    nc.scalar.copy(m1c[:], ps[0][:])
                    t1 = apool.tile([128, 256], F32, tag="t1",
                                    name=f"t1_{nm}_{mt}")
                    nc.vector.tensor_add(t1[:], m1c[:], ps[3][:])
                    nc.vector.tensor_tensor(t1[:], t1[:], ps[4][:], op=SUB)
                    nc.vector.tensor_add(t1[:], t1[:], ps[6][:])
                    t2 = apool.tile([128, 256], F32, tag="t2",
                                    name=f"t2_{nm}_{mt}")
                    nc.scalar.copy(t2[:], ps[2][:])
                    nc.vector.tensor_add(t2[:], t2[:], ps[4][:])
                    t3 = apool.tile([128, 256], F32, tag="t3",
                                    name=f"t3_{nm}_{mt}")
                    nc.scalar.copy(t3[:], ps[1][:])
                    nc.vector.tensor_add(t3[:], t3[:], ps[3][:])
                    t4 = apool.tile([128, 256], F32, tag="t4",
                                    name=f"t4_{nm}_{mt}")
                    nc.vector.tensor_tensor(t4[:], m1c[:], ps[1][:], op=SUB)
                    nc.vector.tensor_add(t4[:], t4[:], ps[2][:])
                    nc.vector.tensor_add(t4[:], t4[:], ps[5][:])
                    for tt, gmt, cols in ((t1, mt, slice(0, 256)),
                                          (t2, mt, slice(256, 512)),
                                          (t3, 8 + mt, slice(0, 256)),
                                          (t4, 8 + mt, slice(256, 512))):
                        nc.scalar.activation(SA[nxt][gmt][:, cols], tt[:],
                                             mybir.ActivationFunctionType.Relu)
                        nc.vector.tensor_scalar_min(SB[nxt][gmt][:, cols],
                                                    tt[:], 0.0)

            for t in range(NITER):
                cur, nxt = t % 2, 1 - t % 2
                sweep(t * (2 * KT * 2), cur, update_bb=True, nm=f"t{t}")
                if t in strset:
                    comp_strassen(strs.index(t), cur, nxt, nm=f"t{t}")
                else:
                    comp_std(stds.index(t), cur, nxt, nm=f"t{t}")
            sweep(NITER * (2 * KT * 2), NITER % 2, update_bb=False, nm="e5")
            nc.sync.dma_start(OUT[0:1, :], best[:])

            if bench_reps > 1:
                loop_ctx.__exit__(None, None, None)

    nc.compile()
    return nc


def _host_inputs(W_upper, W_lower, b_upper, b_lower, ub, lb, strset=STRSET):
    f = np.float32
    W_upper = np.asarray(W_upper, dtype=f)
    W_lower = np.asarray(W_lower, dtype=f)
    b_upper = np.asarray(b_upper, dtype=f)
    b_lower = np.asarray(b_lower, dtype=f)
    ub = np.asarray(ub, dtype=f)
    lb = np.asarray(lb, dtype=f)
    strs = sorted(strset)
    stds = [t for t in range(NITER) if t not in strset]

    # Standard WQ (layer i = NITER-1-t), rows indexed by position in stds.
    if stds:
        iters = [NITER - 1 - t for t in stds]
        A = W_upper[iters].reshape(len(stds), KT, 128, KT, 128)
        B = W_lower[iters].reshape(len(stds), KT, 128, KT, 128)
        T = np.stack([A, B], axis=4)                # [s, k, p, m, h, c]
        WQ = np.ascontiguousarray(
            T.transpose(0, 3, 2, 1, 4, 5).reshape(
                len(stds) * KT * 128, KT * 2 * 128).astype(NPBF16))
    else:
        WQ = np.zeros((KT * 128, KT * 2 * 128), dtype=NPBF16)

    # Strassen WQS: A = [WuT | WlT] [2048, 4096] as 2x2 blocks of
    # [1024, 2048]; 7 host-precomputed A-operands.
    if strs:
        per_s = []
        for t in strs:
            i = NITER - 1 - t
            WuT = W_upper[i].T
            WlT = W_lower[i].T
            A11, A12 = WuT[:1024], WlT[:1024]
            A21, A22 = WuT[1024:], WlT[1024:]
            ops7 = np.stack([A11 + A22, A21 + A22, A11, A22,
                             A11 + A12, A21 - A11, A12 - A22])  # [7,1024,2048]
            O = ops7.reshape(7, 8, 128, KT, 128)     # [op, mt, c, k, p]
            T7 = O.transpose(1, 4, 0, 3, 2)          # [mt, p, op, k, c]
            per_s.append(T7.reshape(8 * 128, 7 * KT * 128))
        WQS = np.ascontiguousarray(
            np.concatenate(per_s, axis=0).astype(NPBF16))
    else:
        WQS = np.zeros((8 * 128, 7 * KT * 128), dtype=NPBF16)

    V = np.zeros((NSWEEP, 2, KT, 128, 2), dtype=f)
    for t in range(NITER):
        i = NITER - 1 - t
        V[t, 0, :, :, 0] = ub[i].reshape(KT, 128)
        V[t, 0, :, :, 1] = b_upper[i].reshape(KT, 128)
        V[t, 1, :, :, 0] = lb[i].reshape(KT, 128)
        V[t, 1, :, :, 1] = b_lower[i].reshape(KT, 128)
    V[NITER, 0, :, :, 0] = ub[0].reshape(KT, 128)
    V[NITER, 1, :, :, 0] = lb[0].reshape(KT, 128)
    VECS = np.ascontiguousarray(
        V.transpose(3, 0, 1, 2, 4).reshape(128, NSWEEP * 2 * KT * 2).astype(NPBF16))

    in_maps = []
    for r in range(N_CORES):
        rows = slice(r * MSH, (r + 1) * MSH)
        Vu = W_upper[L - 1][rows, :].T
        Vln = -W_lower[L - 1][rows, :].T
        st = np.concatenate([Vu, Vln], axis=1)
        SA0 = np.maximum(st, 0).reshape(KT, 128, 512)
        SB0 = np.minimum(st, 0).reshape(KT, 128, 512)
        SAB0 = np.ascontiguousarray(np.stack([SA0, SB0], axis=0).astype(NPBF16))

        BI = np.empty((2, 512), dtype=f)
        BI[0, :256] = ub[L - 1][rows]
        BI[0, 256:] = -lb[L - 1][rows]
        BI[1, :256] = b_upper[L - 1][rows]
        BI[1, 256:] = -b_lower[L - 1][rows]

        in_maps.append(dict(WQ=WQ, WQS=WQS, SAB0=SAB0, VECS=VECS, BI=BI))
    return in_maps


def kernel(W_upper, W_lower, b_upper, b_lower, ub, lb, _trace=False):
    global _CACHED_NC, LAST_RESULTS
    if _CACHED_NC is None:
        _CACHED_NC = _build_nc()
    nc = _CACHED_NC

    in_maps = _host_inputs(W_upper, W_lower, b_upper, b_lower, ub, lb)
    res = run_bass_kernel_spmd(nc, in_maps, list(range(N_CORES)), trace=_trace)
    LAST_RESULTS = res

    best_ub = np.empty(D, dtype=np.float32)
    best_lb = np.empty(D, dtype=np.float32)
    for r in range(N_CORES):
        o = res.results[r]["OUT"].reshape(512)
        best_ub[r * MSH:(r + 1) * MSH] = o[:256]
        best_lb[r * MSH:(r + 1) * MSH] = -o[256:]
    return best_ub, best_lb


# revision 13
# speedup vs baseline: 2.6171x; 2.6171x over previous
"""Trainium2 Bass kernel for 6-layer CROWN backsubstitution — Strassen variant.

Same structure as kernel.py (row-sharded transposed state, masked SA/SB
tiles, bf16 matmuls, fp32 PSUM) but composition iterations in STRSET use
a single-level Strassen split of the [2048, 4096] @ [4096, 512] block
product: 2x2 blocks over (out-rows x contraction x state-cols), 7
M-products of [1024, 2048] @ [2048, 256] instead of 8 -> 12.5% fewer PE
cycles. A-operand combos are precomputed on host (free); B-combos are 5
elementwise bf16 tile sets computed on DVE; C-assembly runs on ACT/DVE
against the one-PSUM-operand rule.
"""

import ml_dtypes
import numpy as np

import concourse.bass as bass
import concourse.bacc as bacc
import concourse.tile as tile
from concourse import mybir
from concourse.bass_utils import run_bass_kernel_spmd

L = 6
D = 2048
N_CORES = 8
MSH = D // N_CORES          # 256 rows per core
KT = D // 128               # 16 k-tiles
NSWEEP = L
NITER = L - 1
STRSET = frozenset({0, 1, 2, 3, 4})   # iterations using Strassen

F32 = mybir.dt.float32
BF16 = mybir.dt.bfloat16
NPBF16 = ml_dtypes.bfloat16
SUB = mybir.AluOpType.subtract

LAST_RESULTS = None
_CACHED_NC = None


def _build_nc(bench_reps=1, strset=STRSET, wch_bufs=10, combo_bufs=1,
              gps_combos=False):
    strs = sorted(strset)
    stds = [t for t in range(NITER) if t not in strset]
    nc = bacc.Bacc("TRN2", target_bir_lowering=False, debug=False,
                   num_devices=N_CORES)

    # Standard-iteration weights (chunk for (t_pos, m) at rows
    # [(t_pos*KT + m)*128, +128)); dummy 1-chunk if unused.
    wq_rows = max(len(stds), 1) * KT * 128
    WQ = nc.dram_tensor("WQ", [wq_rows, KT * 2 * 128], BF16,
                        kind="ExternalInput").ap()
    # Strassen weights: row block (s*8 + mt)*128, col (op*16 + k)*128 + c
    wqs_rows = max(len(strs), 1) * 8 * 128
    WQS = nc.dram_tensor("WQS", [wqs_rows, 7 * KT * 128], BF16,
                         kind="ExternalInput").ap()
    SAB0 = nc.dram_tensor("SAB0", [2, KT, 128, 512], BF16,
                          kind="ExternalInput").ap()
    VECS = nc.dram_tensor("VECS", [128, NSWEEP * 2 * KT * 2], BF16,
                          kind="ExternalInput").ap()
    BI = nc.dram_tensor("BI", [2, 512], F32, kind="ExternalInput").ap()
    OUT = nc.dram_tensor("OUT", [1, 512], F32, kind="ExternalOutput").ap()

    with tile.TileContext(nc) as tc:
        with (
            tc.tile_pool(name="state", bufs=1) as state_pool,
            tc.tile_pool(name="wchunk", bufs=wch_bufs) as wpool,
            tc.tile_pool(name="combo", bufs=combo_bufs) as cpool,
            tc.tile_pool(name="small", bufs=1) as spool,
            tc.tile_pool(name="tmp", bufs=2) as tpool,
            tc.tile_pool(name="asm", bufs=3) as apool,
            tc.tile_pool(name="qpsum", bufs=7, space="PSUM") as qpool,
            tc.tile_pool(name="vpsum", bufs=1, space="PSUM") as vpool,
        ):
            if bench_reps > 1:
                loop_ctx = tc.For_i(0, bench_reps, 1)
                loop_ctx.__enter__()

            SA = [[state_pool.tile([128, 512], BF16, tag=f"sa{s}_{k}",
                                   name=f"sa{s}_{k}")
                   for k in range(KT)] for s in range(2)]
            SB = [[state_pool.tile([128, 512], BF16, tag=f"sb{s}_{k}",
                                   name=f"sb{s}_{k}")
                   for k in range(KT)] for s in range(2)]

            vecs = spool.tile([128, NSWEEP * 2 * KT * 2], BF16, tag="vecs")
            nc.scalar.dma_start(vecs[:], VECS[:])
            best = spool.tile([1, 512], F32, tag="best")
            nc.scalar.dma_start(best[:], BI[0:1, :])
            bb = spool.tile([1, 512], F32, tag="bb")
            nc.scalar.dma_start(bb[:], BI[1:2, :])

            for k in range(KT):
                nc.sync.dma_start(SA[0][k][:], SAB0[0, k])
                nc.scalar.dma_start(SB[0][k][:], SAB0[1, k])

            def sweep(vbase, cur, update_bb, nm):
                """Concretization sweep, 4-way PE column tiling."""
                pv = vpool.tile([128, 512], F32, tag="pv", name=f"pv_{nm}")
                for s in range(8):
                    for g in range(4):
                        side = SA if g < 2 else SB
                        ab = 0 if g < 2 else 1
                        k = (g % 2) * 8 + s
                        j = vbase + (ab * KT + k) * 2
                        nc.tensor.matmul(pv[32 * g:32 * g + 2, :],
                                         vecs[:, j:j + 2], side[cur][k][:],
                                         start=(s == 0), stop=(s == 7),
                                         tile_position=(0, 32 * g))
                s2 = tpool.tile([2, 512], F32, tag="s2", name=f"s2_{nm}")
                nc.scalar.copy(s2[:], pv[0:2, :])
                nc.vector.tensor_add(s2[:], s2[:], pv[32:34, :])
                nc.vector.tensor_add(s2[:], s2[:], pv[64:66, :])
                nc.vector.tensor_add(s2[:], s2[:], pv[96:98, :])
                curt = tpool.tile([1, 512], F32, tag="curt", name=f"curt_{nm}")
                nc.vector.tensor_add(curt[:], s2[0:1, :], bb[:])
                nc.vector.tensor_tensor(best[:], best[:], curt[:],
                                        op=mybir.AluOpType.min)
                if update_bb:
                    bdel = tpool.tile([1, 512], F32, tag="bdel",
                                      name=f"bdel_{nm}")
                    nc.scalar.dma_start(bdel[:], s2[1:2, :])
                    nc.vector.tensor_add(bb[:], bb[:], bdel[:])

            def comp_std(t_pos, cur, nxt, nm):
                """Standard composition: 16 chains of 32 N=512 matmuls."""
                row_base = t_pos * KT * 128
                for m in range(KT):
                    wch = wpool.tile([128, KT * 2 * 128], BF16, tag="wch",
                                     name=f"wch_{nm}_{m}")
                    eng = nc.sync if m % 2 == 0 else nc.scalar
                    eng.dma_start(wch[:],
                                  WQ[row_base + m * 128:row_base + (m + 1) * 128, :])
                    ps = qpool.tile([128, 512], F32, tag="qp512",
                                    name=f"qp_{nm}_{m}")
                    for k in range(KT):
                        nc.tensor.matmul(
                            ps[:], wch[:, (2 * k) * 128:(2 * k + 1) * 128],
                            SA[cur][k][:], start=(k == 0), stop=False)
                        nc.tensor.matmul(
                            ps[:], wch[:, (2 * k + 1) * 128:(2 * k + 2) * 128],
                            SB[cur][k][:], start=False, stop=(k == KT - 1))
                    nc.scalar.activation(SA[nxt][m][:], ps[:],
                                         mybir.ActivationFunctionType.Relu)
                    nc.vector.tensor_scalar_min(SB[nxt][m][:], ps[:], 0.0)

            def comp_strassen(s_idx, cur, nxt, nm):
                """Strassen composition: per mt, 7 chains of 16 N=256 matmuls.

                B-blocks (bf16): B11=SA[:, :256], B12=SA[:, 256:],
                B21=SB[:, :256], B22=SB[:, 256:].
                """
                # B-combos on DVE, one op per (type, k).
                CB = {}
                for cn, fa, fb, op in (
                    ("c1", lambda k: SA[cur][k][:, 0:256],
                     lambda k: SB[cur][k][:, 256:512], None),      # B11+B22
                    ("c3", lambda k: SA[cur][k][:, 256:512],
                     lambda k: SB[cur][k][:, 256:512], SUB),       # B12-B22
                    ("c4", lambda k: SB[cur][k][:, 0:256],
                     lambda k: SA[cur][k][:, 0:256], SUB),         # B21-B11
                    ("c6", lambda k: SA[cur][k][:, 0:256],
                     lambda k: SA[cur][k][:, 256:512], None),      # B11+B12
                    ("c7", lambda k: SB[cur][k][:, 0:256],
                     lambda k: SB[cur][k][:, 256:512], None),      # B21+B22
                ):
                    tiles = []
                    for k in range(KT):
                        cb = cpool.tile([128, 256], BF16, tag=f"{cn}_{k}",
                                        name=f"{cn}_{nm}_{k}")
                        if op is None:
                            eng = (nc.gpsimd if (gps_combos and
                                                 cn in ("c6", "c7"))
                                   else nc.vector)
                            eng.tensor_add(cb[:], fa(k), fb(k))
                        else:
                            nc.vector.tensor_tensor(cb[:], fa(k), fb(k), op=op)
                        tiles.append(cb)
                    CB[cn] = tiles

                def rhs(op, k):
                    return (CB["c1"][k][:], SA[cur][k][:, 0:256],
                            CB["c3"][k][:], CB["c4"][k][:],
                            SB[cur][k][:, 256:512], CB["c6"][k][:],
                            CB["c7"][k][:])[op]

                row0 = s_idx * 8 * 128
                for mt in range(8):
                    wops = []
                    for op in range(7):
                        w = wpool.tile([128, KT * 128], BF16, tag="wop",
                                       name=f"w_{nm}_{mt}_{op}")
                        eng = nc.sync if (mt * 7 + op) % 2 == 0 else nc.scalar
                        r = row0 + mt * 128
                        eng.dma_start(
                            w[:], WQS[r:r + 128,
                                      op * KT * 128:(op + 1) * KT * 128])
                        wops.append(w)
                    # Two M-products per PSUM bank (disjoint column halves,
                    # sequential accumulation groups).
                    psp = [qpool.tile([128, 512], F32, tag="qp512",
                                      name=f"mp_{nm}_{mt}_{j}")
                           for j in range(4)]
                    ps = [psp[i // 2][:, (i % 2) * 256:(i % 2 + 1) * 256]
                          for i in range(7)]
                    for op in range(7):
                        for k in range(KT):
                            nc.tensor.matmul(ps[op],
                                             wops[op][:, k * 128:(k + 1) * 128],
                                             rhs(op, k), start=(k == 0),
                                             stop=(k == KT - 1))
                    # Assembly: C11=M1+M4-M5+M7  C12=M3+M5  C21=M2+M4
                    #           C22=M1-M2+M3+M6      (ps[i] = M_{i+1})
                    m1c = apool.tile([128, 256], F32, tag="m1c",
                                     name=f"m1c_{nm}_{mt}")
                    nc.scalar.copy(m1c[:], ps[0][:])
                    t1 = apool.tile([128, 256], F32, tag="t1",
                                    name=f"t1_{nm}_{mt}")
                    nc.vector.tensor_add(t1[:], m1c[:], ps[3][:])
                    nc.vector.tensor_tensor(t1[:], t1[:], ps[4][:], op=SUB)
                    nc.vector.tensor_add(t1[:], t1[:], ps[6][:])
                    t2 = apool.tile([128, 256], F32, tag="t2",
                                    name=f"t2_{nm}_{mt}")
                    nc.scalar.copy(t2[:], ps[2][:])
                    nc.vector.tensor_add(t2[:], t2[:], ps[4][:])
                    t3 = apool.tile([128, 256], F32, tag="t3",
                                    name=f"t3_{nm}_{mt}")
                    nc.scalar.copy(t3[:], ps[1][:])
                    nc.vector.tensor_add(t3[:], t3[:], ps[3][:])
                    t4 = apool.tile([128, 256], F32, tag="t4",
                                    name=f"t4_{nm}_{mt}")
                    nc.vector.tensor_tensor(t4[:], m1c[:], ps[1][:], op=SUB)
                    nc.vector.tensor_add(t4[:], t4[:], ps[2][:])
                    nc.vector.tensor_add(t4[:], t4[:], ps[5][:])
                    for tt, gmt, cols in ((t1, mt, slice(0, 256)),
                                          (t2, mt, slice(256, 512)),
                                          (t3, 8 + mt, slice(0, 256)),
                                          (t4, 8 + mt, slice(256, 512))):
                        nc.scalar.activation(SA[nxt][gmt][:, cols], tt[:],
                                             mybir.ActivationFunctionType.Relu)
                        nc.vector.tensor_scalar_min(SB[nxt][gmt][:, cols],
                                                    tt[:], 0.0)

            for t in range(NITER):
                cur, nxt = t % 2, 1 - t % 2
                sweep(t * (2 * KT * 2), cur, update_bb=True, nm=f"t{t}")
                if t in strset:
                    comp_strassen(strs.index(t), cur, nxt, nm=f"t{t}")
                else:
                    comp_std(stds.index(t), cur, nxt, nm=f"t{t}")
            sweep(NITER * (2 * KT * 2), NITER % 2, update_bb=False, nm="e5")
            nc.sync.dma_start(OUT[0:1, :], best[:])

            if bench_reps > 1:
                loop_ctx.__exit__(None, None, None)

    nc.compile()
    return nc


def _host_inputs(W_upper, W_lower, b_upper, b_lower, ub, lb, strset=STRSET):
    f = np.float32
    W_upper = np.asarray(W_upper, dtype=f)
    W_lower = np.asarray(W_lower, dtype=f)
    b_upper = np.asarray(b_upper, dtype=f)
    b_lower = np.asarray(b_lower, dtype=f)
    ub = np.asarray(ub, dtype=f)
    lb = np.asarray(lb, dtype=f)
    strs = sorted(strset)
    stds = [t for t in range(NITER) if t not in strset]

    # Standard WQ (layer i = NITER-1-t), rows indexed by position in stds.
    if stds:
        iters = [NITER - 1 - t for t in stds]
        A = W_upper[iters].reshape(len(stds), KT, 128, KT, 128)
        B = W_lower[iters].reshape(len(stds), KT, 128, KT, 128)
        T = np.stack([A, B], axis=4)                # [s, k, p, m, h, c]
        WQ = np.ascontiguousarray(
            T.transpose(0, 3, 2, 1, 4, 5).reshape(
                len(stds) * KT * 128, KT * 2 * 128).astype(NPBF16))
    else:
        WQ = np.zeros((KT * 128, KT * 2 * 128), dtype=NPBF16)

    # Strassen WQS: A = [WuT | WlT] [2048, 4096] as 2x2 blocks of
    # [1024, 2048]; 7 host-precomputed A-operands.
    if strs:
        per_s = []
        for t in strs:
            i = NITER - 1 - t
            WuT = W_upper[i].T
            WlT = W_lower[i].T
            A11, A12 = WuT[:1024], WlT[:1024]
            A21, A22 = WuT[1024:], WlT[1024:]
            ops7 = np.stack([A11 + A22, A21 + A22, A11, A22,
                             A11 + A12, A21 - A11, A12 - A22])  # [7,1024,2048]
            O = ops7.reshape(7, 8, 128, KT, 128)     # [op, mt, c, k, p]
            T7 = O.transpose(1, 4, 0, 3, 2)          # [mt, p, op, k, c]
            per_s.append(T7.reshape(8 * 128, 7 * KT * 128))
        WQS = np.ascontiguousarray(
            np.concatenate(per_s, axis=0).astype(NPBF16))
    else:
        WQS = np.zeros((8 * 128, 7 * KT * 128), dtype=NPBF16)

    V = np.zeros((NSWEEP, 2, KT, 128, 2), dtype=f)
    for t in range(NITER):
        i = NITER - 1 - t
        V[t, 0, :, :, 0] = ub[i].reshape(KT, 128)
        V[t, 0, :, :, 1] = b_upper[i].reshape(KT, 128)
        V[t, 1, :, :, 0] = lb[i].reshape(KT, 128)
        V[t, 1, :, :, 1] = b_lower[i].reshape(KT, 128)
    V[NITER, 0, :, :, 0] = ub[0].reshape(KT, 128)
    V[NITER, 1, :, :, 0] = lb[0].reshape(KT, 128)
    VECS = np.ascontiguousarray(
        V.transpose(3, 0, 1, 2, 4).reshape(128, NSWEEP * 2 * KT * 2).astype(NPBF16))

    in_maps = []
    for r in range(N_CORES):
        rows = slice(r * MSH, (r + 1) * MSH)
        Vu = W_upper[L - 1][rows, :].T
        Vln = -W_lower[L - 1][rows, :].T
        st = np.concatenate([Vu, Vln], axis=1)
        SA0 = np.maximum(st, 0).reshape(KT, 128, 512)
        SB0 = np.minimum(st, 0).reshape(KT, 128, 512)
        SAB0 = np.ascontiguousarray(np.stack([SA0, SB0], axis=0).astype(NPBF16))

        BI = np.empty((2, 512), dtype=f)
        BI[0, :256] = ub[L - 1][rows]
        BI[0, 256:] = -lb[L - 1][rows]
        BI[1, :256] = b_upper[L - 1][rows]
        BI[1, 256:] = -b_lower[L - 1][rows]

        in_maps.append(dict(WQ=WQ, WQS=WQS, SAB0=SAB0, VECS=VECS, BI=BI))
    return in_maps


def kernel(W_upper, W_lower, b_upper, b_lower, ub, lb, _trace=False):
    global _CACHED_NC, LAST_RESULTS
    if _CACHED_NC is None:
        _CACHED_NC = _build_nc()
    nc = _CACHED_NC

    in_maps = _host_inputs(W_upper, W_lower, b_upper, b_lower, ub, lb)
    res = run_bass_kernel_spmd(nc, in_maps, list(range(N_CORES)), trace=_trace)
    LAST_RESULTS = res

    best_ub = np.empty(D, dtype=np.float32)
    best_lb = np.empty(D, dtype=np.float32)
    for r in range(N_CORES):
        o = res.results[r]["OUT"].reshape(512)
        best_ub[r * MSH:(r + 1) * MSH] = o[:256]
        best_lb[r * MSH:(r + 1) * MSH] = -o[256:]
    return best_ub, best_lb
